# revision 38
# baseline (speedup 1.0000x reference)
"""Trainium2 Bass kernel for batched multi-head attention.

Problem: B=8, H=8, S=2048, D=64 f32 attention,
  out = softmax(Q K^T / 64**0.25) V  per (b, h).

Sharding: the 64 (b,h) pairs are split 8-per-core across the 8 NeuronCores
(pure data/head parallelism, no collectives).

v3 design (v1 baseline ~315us was ACT-bound on softmax exp):
  - exp split across engines: 5/8 of k-chunk pairs use exact ACT exp, 3/8
    use a Schraudolph fast exp on the otherwise-idle Vector engine - one
    tensor_scalar computing int16(bf16-bits) = scores*A + B, reinterpreted
    as bf16 (end-to-end rel err ~1e-2 vs 2e-2 tolerance).
  - QK strip pairs (K=64) carry explicit tile_position row tiling so both
    64-row matmuls stream concurrently through the PE.
  - AV chunks of stage i-1 are interleaved between QK pairs of stage i in
    PE issue order, so the PE never drains while exp production paces QK
    through the 3-deep PSUM pool.
  - All input loads are issued once at t=0 (SBUF holds all 8 heads,
    ~162KB/partition): DMA descriptor issue costs ~600ns of sequencer time
    each, so mid-flight load bursts at head boundaries stall the queues.
  - Softmax denominators (ones-column of the AV stationary) are processed
    once per head, not per slab: po -> SBUF copy per slab (also frees the
    PSUM bank), then one row DMA -> [128,16] spread -> reciprocal -> DRAM
    -> [64,S] stride-0 broadcast -> normalize multiply on GPSIMD. Each
    link is issued on a queue whose head-of-line dependency is just-met
    (DMA issue split between the GPSIMD and Sync queues), with multiple
    stages of slack so no strict-FIFO engine queue ever parks.
"""
import sys

sys.path.insert(0, "/opt/trn_rl_repo")

from contextlib import ExitStack

import ml_dtypes
import numpy as np

import concourse.bass as bass
import concourse.tile as tile
from concourse import bacc, mybir
from concourse.bass_utils import run_bass_kernel_spmd

B, H, S, D = 8, 8, 2048, 64
N_CORES = 8
HPC = B * H // N_CORES  # heads per core = 8
SCALE = 1.0 / (D**0.5) ** 0.5  # 1 / 64**0.25
PCHUNK = 128  # k rows per chunk
NCHUNK = S // PCHUNK  # 16
NPAIR = NCHUNK // 2  # 8 chunk pairs per slab
SLAB = 512  # q columns per slab
NSLAB = S // SLAB  # 4
BF16 = mybir.dt.bfloat16
F32 = mybir.dt.float32
I16 = mybir.dt.int16

# Schraudolph fast-exp on DVE: bf16 bits of exp(s*SCALE) ~= s*A1 + B1
LOG2E = 1.4426950408889634
FEXP_C = 0.06  # sawtooth centering, tuned in numerics_sim.py
A1 = 128.0 * LOG2E * SCALE
B1 = 128.0 * (127.0 - FEXP_C)
DVE_PAIRS = (2, 5, 7)  # pairs computed on DVE; rest on ACT

_COMPILED = {}


def build_kernel():
    nc = bacc.Bacc("TRN2", target_bir_lowering=False, debug=False)
    qt = nc.dram_tensor("q_t", [HPC, D, S], BF16, kind="ExternalInput").ap()
    kt = nc.dram_tensor("k_t", [HPC, D, S], BF16, kind="ExternalInput").ap()
    v = nc.dram_tensor("v", [HPC, S, D], BF16, kind="ExternalInput").ap()
    out = nc.dram_tensor("out_t", [HPC, D, S], F32, kind="ExternalOutput").ap()
    # DRAM bounce buffers for the cross-partition softmax-denominator move
    s_dram = nc.dram_tensor("s_scratch", [HPC, S], F32).ap()
    r_dram = nc.dram_tensor("r_scratch", [HPC, S], F32).ap()

    with tile.TileContext(nc) as tc, ExitStack() as ctx:
        qk_pool = ctx.enter_context(tc.tile_pool(name="qk", bufs=1))
        v_pool = ctx.enter_context(tc.tile_pool(name="vp", bufs=1))
        exp_pool = ctx.enter_context(tc.tile_pool(name="exp", bufs=2))
        ou_pool = ctx.enter_context(tc.tile_pool(name="ou", bufs=3))
        fin_pool = ctx.enter_context(tc.tile_pool(name="fin", bufs=2))
        small_pool = ctx.enter_context(tc.tile_pool(name="small", bufs=2))
        const_pool = ctx.enter_context(tc.tile_pool(name="const", bufs=1))
        # PSUM: psqk_act 2 x 2 banks + psqk_dve 2 x 1 bank + psav 2 x 1
        # bank = 8 banks exactly.  ACT-pair and DVE-chunk score tiles live
        # in separate pools so each exp engine's QK->exp->QK recycling
        # chain only ever waits on its own exps (no cross-engine bubbles).
        psqk_act_pool = ctx.enter_context(
            tc.tile_pool(name="psqka", bufs=2, space="PSUM")
        )
        psqk_dve_pool = ctx.enter_context(
            tc.tile_pool(name="psqkd", bufs=2, space="PSUM")
        )
        psav_pool = ctx.enter_context(
            tc.tile_pool(name="psav", bufs=2, space="PSUM")
        )

        zbias = const_pool.tile([128, 1], F32)
        nc.vector.memset(zbias[:], 0.0)
        # warm the ACT exp table at t=0 so its ~2.7us load overlaps the
        # first input DMAs instead of delaying the first real exp
        warm = const_pool.tile([128, 1], F32)
        nc.scalar.activation(
            warm[:],
            zbias[:],
            mybir.ActivationFunctionType.Exp,
            bias=zbias[:],
            scale=1.0,
        )
        # warm the GPSIMD tensor_tensor ucode path at t=0 as well
        warm2 = const_pool.tile([128, 1], F32)
        nc.gpsimd.tensor_tensor(
            warm2[:], zbias[:], zbias[:], op=mybir.AluOpType.mult
        )

        # ---- all input loads up front (head-major so head 0 lands first);
        # head 0's first q-slab pieces go first so the PE starts in ~2us
        # strip-B column offset for the duplicated copies (a 2KB bank-shift
        # experiment measured slower; keep the aligned layout)
        BOFF = 0
        head_tiles = {}
        for h in range(HPC):
            qt_sb = qk_pool.tile([2 * D, S + BOFF], BF16, tag=f"qt{h}")
            kt_sb = qk_pool.tile([2 * D, S + BOFF], BF16, tag=f"kt{h}")
            pieces = (
                [(0, SLAB), (SLAB, S)] if h == 0 else [(0, S)]
            )
            for lo, hi in pieces:
                cols = slice(lo, hi)
                colsb = slice(BOFF + lo, BOFF + hi)
                nc.sync.dma_start(kt_sb[0:D, cols], kt[h][:, cols])
                nc.sync.dma_start(kt_sb[D : 2 * D, colsb], kt[h][:, cols])
                nc.sync.dma_start(qt_sb[0:D, cols], qt[h][:, cols])
                nc.sync.dma_start(qt_sb[D : 2 * D, colsb], qt[h][:, cols])
            v_aug = v_pool.tile([PCHUNK, NCHUNK, D + 1], BF16, tag=f"v{h}")
            nc.vector.memset(v_aug[:, :, D : D + 1], 1.0)
            nc.sync.dma_start(
                v_aug[:, :, 0:D], v[h].rearrange("(c p) d -> p c d", p=PCHUNK)
            )
            head_tiles[h] = (qt_sb, kt_sb, v_aug)

        st = {}  # stage index -> state
        hd = {}  # head -> state for the per-head denominator chain

        def qk_pair(i, p):
            d = st[i]
            h, s = d["h"], d["s"]
            qt_sb, kt_sb, _ = head_tiles[h]
            cols = slice(s * SLAB, (s + 1) * SLAB)
            if p in DVE_PAIRS:
                # chunk-granular 1-bank tiles + FD=512 fast exp on DVE
                for half in range(2):
                    c = 2 * p + half
                    base = half * D
                    off = BOFF if half else 0
                    ps = psqk_dve_pool.tile([PCHUNK, SLAB], F32, tag="psqkd")
                    nc.tensor.matmul(
                        ps[:],
                        kt_sb[
                            base : base + D,
                            off + c * PCHUNK : off + (c + 1) * PCHUNK,
                        ],
                        qt_sb[base : base + D, off + s * SLAB : off + (s + 1) * SLAB],
                        start=True,
                        stop=True,
                        tile_position=(base, 0),
                    )
                    nc.vector.tensor_scalar(
                        d["expT"][:, c, :].bitcast(I16),
                        ps[:],
                        A1,
                        B1,
                        mybir.AluOpType.mult,
                        mybir.AluOpType.add,
                    )
            else:
                ps = psqk_act_pool.tile([PCHUNK, 2, SLAB], F32, tag="psqka")
                for half in range(2):
                    c = 2 * p + half
                    base = half * D
                    off = BOFF if half else 0
                    nc.tensor.matmul(
                        ps[:, half, :],
                        kt_sb[
                            base : base + D,
                            off + c * PCHUNK : off + (c + 1) * PCHUNK,
                        ],
                        qt_sb[base : base + D, off + s * SLAB : off + (s + 1) * SLAB],
                        start=True,
                        stop=True,
                        tile_position=(base, 0),
                    )
                nc.scalar.activation(
                    d["expT"][:, 2 * p : 2 * p + 2, :],
                    ps[:],
                    mybir.ActivationFunctionType.Exp,
                    bias=zbias[:],
                    scale=SCALE,
                )

        def av_chunks(i, c0, cnt):
            d = st[i]
            _, _, v_aug = head_tiles[d["h"]]
            if d["po"] is None:
                po = psav_pool.tile([D + 1, SLAB], F32, tag="psav")
                d["po"] = po
            for c in range(c0, c0 + cnt):
                nc.tensor.matmul(
                    d["po"][:],
                    v_aug[:, c, :],
                    d["expT"][:, c, :],
                    start=(c == 0),
                    stop=(c == NCHUNK - 1),
                )

        def norm_copy(i):
            # po (unnormalized out + sums row) PSUM -> the head's SBUF
            # accumulator, freeing the PSUM bank
            d = st[i]
            h, s = d["h"], d["s"]
            if h not in hd:
                ou_sb = ou_pool.tile([D + 1, S], F32, tag="ousb")
                hd[h] = {"ou_sb": ou_sb}
            nc.vector.tensor_copy(
                hd[h]["ou_sb"][:, s * SLAB : (s + 1) * SLAB], d["po"][:]
            )
            del st[i]

        def chain_a(h, lo, hi, eng=None):
            # sums row -> DRAM -> [128, n/128] spread tile
            g = hd[h]
            key = (lo, hi)
            (eng or nc.gpsimd).dma_start(
                s_dram[h, lo:hi], g["ou_sb"][D : D + 1, lo:hi]
            )
            sums = small_pool.tile([128, (hi - lo) // 128], F32, tag="sums")
            nc.sync.dma_start(
                sums[:], s_dram[h, lo:hi].rearrange("(c p) -> p c", p=128)
            )
            g.setdefault("sums", {})[key] = sums

        def chain_b(h, lo, hi, eng=None):
            # reciprocal -> DRAM -> [D, n] stride-0 broadcast
            g = hd[h]
            key = (lo, hi)
            rnat = small_pool.tile([128, (hi - lo) // 128], F32, tag="rnat")
            nc.vector.reciprocal(rnat[:], g["sums"][key][:])
            (eng or nc.gpsimd).dma_start(
                r_dram[h, lo:hi].rearrange("(c p) -> p c", p=128), rnat[:]
            )
            r_bc = small_pool.tile([D, hi - lo], F32, tag="rbc")
            nc.sync.dma_start(
                r_bc[:],
                bass.AP(r_dram.tensor, h * S + lo, [[0, D], [1, hi - lo]]),
            )
            g.setdefault("r_bc", {})[key] = r_bc

        def chain_c(h, lo, hi, last):
            # normalize on GPSIMD and store
            g = hd[h]
            key = (lo, hi)
            o_fin = fin_pool.tile([D, hi - lo], F32, tag="ofin")
            nc.gpsimd.tensor_tensor(
                o_fin[:],
                g["ou_sb"][0:D, lo:hi],
                g["r_bc"][key][:],
                op=mybir.AluOpType.mult,
            )
            nc.gpsimd.dma_start(out[h][:, lo:hi], o_fin[:])
            if last:
                del hd[h]

        stages = [(h, s) for h in range(HPC) for s in range(NSLAB)]
        n = len(stages)
        LAST_H = HPC - 1
        for i in range(n + 4):
            if i < n:
                h, s = stages[i]
                expT = exp_pool.tile([PCHUNK, NCHUNK, SLAB], BF16, tag="expT")
                st[i] = {"h": h, "s": s, "po": None, "expT": expT}
                prev = i - 1 if i >= 1 else None
                for p in range(4):
                    qk_pair(i, p)
                if prev is not None:
                    av_chunks(prev, 0, 8)
                for p in range(4, NPAIR):
                    qk_pair(i, p)
                if prev is not None:
                    av_chunks(prev, 8, 8)
            elif i == n:
                av_chunks(n - 1, 0, 8)
                av_chunks(n - 1, 8, 8)
            # per-head denominator chain, one link per stage slot, running
            # during the next head's first stages.  The final head drains
            # per-slab so its chain overlaps its own compute.
            if i >= 5 and (i - 5) % NSLAB == 0 and (i - 5) // NSLAB < LAST_H:
                chain_a((i - 5) // NSLAB, 0, S)
            if i >= 6 and (i - 6) % NSLAB == 0 and (i - 6) // NSLAB < LAST_H:
                chain_b((i - 6) // NSLAB, 0, S)
            if i >= 7 and (i - 7) % NSLAB == 0 and (i - 7) // NSLAB < LAST_H:
                chain_c((i - 7) // NSLAB, 0, S, True)
            # last head, per-slab: copy(LAST_H, s) lands at slot 4*LAST_H+s+1
            base = NSLAB * LAST_H
            if i - 1 >= 0 and i - 1 in st and st[i - 1]["po"] is not None:
                norm_copy(i - 1)
            if i >= base + 1 and i - base - 1 < NSLAB:
                sl = i - base - 1
                chain_a(LAST_H, sl * SLAB, (sl + 1) * SLAB, eng=nc.sync)
            if i >= base + 2 and i - base - 2 < NSLAB:
                sl = i - base - 2
                chain_b(LAST_H, sl * SLAB, (sl + 1) * SLAB, eng=nc.sync)
            if i >= base + 3 and i - base - 3 < NSLAB:
                sl = i - base - 3
                chain_c(
                    LAST_H, sl * SLAB, (sl + 1) * SLAB, sl == NSLAB - 1
                )
    nc.compile()
    return nc


def _get_compiled():
    if "nc" not in _COMPILED:
        _COMPILED["nc"] = build_kernel()
    return _COMPILED["nc"]


def kernel(query, key, value, _want_results=False):
    nc = _get_compiled()
    q = np.asarray(query).reshape(B * H, S, D)
    k = np.asarray(key).reshape(B * H, S, D)
    v = np.asarray(value).reshape(B * H, S, D)
    in_maps = []
    for c in range(N_CORES):
        sl = slice(c * HPC, (c + 1) * HPC)
        in_maps.append(
            {
                "q_t": np.ascontiguousarray(q[sl].transpose(0, 2, 1)).astype(
                    ml_dtypes.bfloat16
                ),
                "k_t": np.ascontiguousarray(k[sl].transpose(0, 2, 1)).astype(
                    ml_dtypes.bfloat16
                ),
                "v": np.ascontiguousarray(v[sl]).astype(ml_dtypes.bfloat16),
            }
        )
    res = run_bass_kernel_spmd(nc, in_maps, core_ids=list(range(N_CORES)))
    out = np.concatenate(
        [
            res.results[c]["out_t"].transpose(0, 2, 1).reshape(1, HPC, S, D)
            for c in range(N_CORES)
        ],
        axis=0,
    ).reshape(B, H, S, D)
    if _want_results:
        return out, res
    return out


if __name__ == "__main__":
    rng = np.random.default_rng(0)
    q = rng.standard_normal((B, H, S, D), dtype=np.float32)
    k = rng.standard_normal((B, H, S, D), dtype=np.float32)
    v = rng.standard_normal((B, H, S, D), dtype=np.float32)
    o = kernel(q, k, v)
    print("kernel output", o.shape, o.dtype)
